# revision 89
# baseline (speedup 1.0000x reference)
"""Cross-attention Trainium2 kernel (Bass/Tile), SPMD over 8 NeuronCores.

Problem (hardcoded): x[4,4096,1024], context[4,512,768], Wq[1024,1024],
Wk[768,1024], Wv[768,1024], Wo[1024,1024], bo[1024]; 16 heads, dim 64.
    q = x@Wq; k = ctx@Wk; v = ctx@Wv (per-head 64)
    out = softmax(q k^T / 8) v;  y = out@Wo + bo
Sharding: core i -> (batch b = i//2, query half = i%2, 2048 rows), all 16
heads per core. No collectives; host concatenates the 8 output shards.

Device dataflow. The cost model charges a matmul output-free-size rows at
1 cyc/row for bf16 at any size, and for f32r only when free >= 256 — so
every matmul keeps free >= 256 in f32r, or uses a bf16 moving operand:
    QT[d,n]   = Wq^T x^T          (f32r psum -> f32r qt tiles)
    KT[d,m]   = Wk^T ctx^T        (f32r kt tiles)
    V[m,d]    = ctx Wv            (bf16 v tiles; +ones column per head)
    ET[m,n]   = exp(KT_h^T QT_h)  (f32r et tiles, ACT exp)
    PV        = ET^T chunks -> out_nd[n, 65]  (bf16 moving V, free=65;
                col 64 = softmax denominator; psum [n-chunk, 4*65])
    norm      = DVE per-partition scale (recip of col 64) during the
                psum->sbuf copy -> ot_nd[n, d-pair] bf16
    T         = PE transpose (bf16 identity) -> otT[d-pair, n] = Wo lhsT
    y[n,c]    = otT^T Wo + bo
Softmax max-subtraction is skipped: scores ~ N(0,1), exp safe in fp32.
The 1/8 scale is folded into Wq on the host; x/ctx/weights ship as bf16.
"""

import numpy as np
import ml_dtypes

import concourse.bass as bass
import concourse.mybir as mybir
import concourse.tile as tile
from concourse import bacc, masks
from concourse.bass_utils import run_bass_kernel_spmd

F32 = mybir.dt.float32
F32R = mybir.dt.float32r
BF16 = mybir.dt.bfloat16

B, N, C = 4, 4096, 1024
M, CC = 512, 768
H, D = 16, 64
INNER = H * D          # 1024
NPC = N // 2           # 2048 query rows per core
NT = NPC // 512        # 4 n-tiles of 512
NCHUNK_Q = C // 128    # 8 contraction chunks for Q proj
NCHUNK_K = CC // 128   # 6 contraction chunks for K/V proj
NPAIR = H // 2         # 8 head pairs (2 heads stacked per 128 partitions)
NMC = M // 128         # 4 key chunks
VBLK = D + 1           # 65: V columns + ones column per head


def build_nc() -> bass.Bass:
    nc = bacc.Bacc("TRN2", target_bir_lowering=False, debug=False, num_devices=8)

    xT = nc.dram_tensor("xT", [C, NPC], BF16, kind="ExternalInput")
    ctxT = nc.dram_tensor("ctxT", [CC, M], BF16, kind="ExternalInput")
    wq = nc.dram_tensor("wq", [C, INNER], BF16, kind="ExternalInput")
    wk = nc.dram_tensor("wk", [CC, INNER], BF16, kind="ExternalInput")
    wv = nc.dram_tensor("wv", [CC, INNER], BF16, kind="ExternalInput")
    wo = nc.dram_tensor("wo", [INNER, C], BF16, kind="ExternalInput")
    bo = nc.dram_tensor("bo", [1, C], F32R, kind="ExternalInput")
    y = nc.dram_tensor("y", [NPC, C], BF16, kind="ExternalOutput")

    with tile.TileContext(nc) as tc:
        with (
            tc.tile_pool(name="persist", bufs=1) as pp,
            tc.tile_pool(name="psA", bufs=2, space="PSUM") as ps_a,
            tc.tile_pool(name="psSC", bufs=2, space="PSUM") as ps_sc,
            tc.tile_pool(name="psPV", bufs=2, space="PSUM") as ps_pv,
        ):
            # ---- persistent SBUF ----
            wq_sb = pp.tile([128, NCHUNK_Q * INNER], BF16)
            wo_sb = pp.tile([128, NCHUNK_Q * C], BF16)
            kt_sb = pp.tile([128, NPAIR * M], F32R)
            v_sb = pp.tile([128, NMC * H * VBLK], BF16)
            ident = pp.tile([128, 128], BF16)
            ones_f32 = pp.tile([128, 128], F32)
            ones_r = pp.tile([1, 128], F32R)
            ones_bf = pp.tile([1, 128], BF16)
            bo_sb = pp.tile([128, C], F32)
            bo_row = pp.tile([1, C], F32R)
            bo_bf = pp.tile([1, C], BF16)

            nc.vector.memset(ones_f32[:], 1.0)
            nc.scalar.copy(out=ones_r[:], in_=ones_f32[0:1, 0:128])
            nc.scalar.copy(out=ones_bf[:], in_=ones_f32[0:1, 0:128])
            # ones column (col 64) of every (mc, head) block of V
            nc.scalar.copy(
                out=v_sb[:].rearrange("p (b q) -> p b q", q=VBLK)[:, :, D : D + 1],
                in_=ones_f32[:, 0 : NMC * H].rearrange("p (b q) -> p b q", q=1),
            )
            masks.make_identity(nc, ident[:])

            with tc.tile_pool(name="setup", bufs=1) as sp:
                wk_sb = sp.tile([128, NCHUNK_K * INNER], BF16)
                wv_sb = sp.tile([128, NCHUNK_K * INNER], BF16)
                ctx_sb = sp.tile([128, NCHUNK_K * M], BF16)

                # DMA priority order: first K-proj chunk pair split out so
                # the PE can start ~2us in, the rest batched (one HWDGE
                # issue each): wk/ctx -> wv -> wq -> x0 -> x1 -> wo -> bo.
                for c in range(2):
                    nc.sync.dma_start(
                        out=wk_sb[:, c * INNER : (c + 1) * INNER],
                        in_=wk[c * 128 : (c + 1) * 128, :],
                    )
                    nc.sync.dma_start(
                        out=ctx_sb[:, c * M : (c + 1) * M],
                        in_=ctxT[c * 128 : (c + 1) * 128, :],
                    )
                for c0 in (2, 4):
                    nc.sync.dma_start(
                        out=wk_sb[:, c0 * INNER : (c0 + 2) * INNER].rearrange(
                            "p (c q) -> p c q", q=INNER
                        ),
                        in_=wk[c0 * 128 : (c0 + 2) * 128, :].rearrange(
                            "(c p) q -> p c q", p=128
                        ),
                    )
                    nc.sync.dma_start(
                        out=ctx_sb[:, c0 * M : (c0 + 2) * M].rearrange(
                            "p (c q) -> p c q", q=M
                        ),
                        in_=ctxT[c0 * 128 : (c0 + 2) * 128, :].rearrange(
                            "(c p) q -> p c q", p=128
                        ),
                    )
                nc.sync.dma_start(
                    out=wv_sb[:].rearrange("p (c q) -> p c q", q=INNER),
                    in_=wv[:, :].rearrange("(c p) q -> p c q", p=128),
                )
                nc.sync.dma_start(
                    out=wq_sb[:].rearrange("p (c q) -> p c q", q=INNER),
                    in_=wq[:, :].rearrange("(c p) q -> p c q", p=128),
                )

                with (
                    tc.tile_pool(name="xt", bufs=3) as xp,
                    tc.tile_pool(name="qt", bufs=12) as qp,
                    tc.tile_pool(name="et", bufs=12) as ep,
                    tc.tile_pool(name="ond", bufs=4) as onp,
                    tc.tile_pool(name="otT", bufs=16) as otp,
                    tc.tile_pool(name="rr", bufs=6) as rp,
                    tc.tile_pool(name="ysb", bufs=8) as yp,
                ):
                    xt_t = {}
                    qt_t = {}
                    otT_t = {}

                    def emit_x_dma(nt):
                        t = xp.tile([128, NCHUNK_Q * 512], BF16, tag="xt", name=f"xt{nt}")
                        nc.sync.dma_start(
                            out=t[:].rearrange("p (c q) -> p c q", q=512),
                            in_=xT[:, nt * 512 : (nt + 1) * 512].rearrange(
                                "(c p) q -> p c q", p=128
                            ),
                        )
                        xt_t[nt] = t

                    emit_x_dma(0)
                    emit_x_dma(1)
                    nc.sync.dma_start(
                        out=wo_sb[:].rearrange("p (c q) -> p c q", q=C),
                        in_=wo[:, :].rearrange("(c p) q -> p c q", p=128),
                    )
                    # bo rides the Pool engine's SWDGE path so it skips the
                    # HWDGE queue entirely and lands ~3us in
                    nc.gpsimd.dma_start(out=bo_row[:], in_=bo[:, :])

                    def emit_bias_bcast():
                        # bias broadcast to all partitions via PE outer product
                        for cg in range(2):
                            bps = ps_a.tile([128, 512], F32, tag="psA")
                            nc.tensor.matmul(
                                bps[:],
                                ones_r[0:1, 0:128],
                                bo_row[0:1, cg * 512 : (cg + 1) * 512],
                                start=True,
                                stop=True,
                            )
                            nc.scalar.copy(
                                out=bo_sb[:, cg * 512 : (cg + 1) * 512], in_=bps[:]
                            )
                        nc.scalar.copy(out=bo_bf[:], in_=bo_row[:])

                    # KT per head pair: [128 (2 heads d), 512 m]. The first
                    # group interleaves the bias outer-products into its
                    # chunk-arrival waits (bias only needs bo via SWDGE).
                    for j in range(NPAIR):
                        kps = ps_sc.tile([128, 1024], F32, tag="sc")
                        for c in range(NCHUNK_K):
                            if j == 0 and c == 2:
                                emit_bias_bcast()
                            nc.tensor.matmul(
                                kps[:, 0:M],
                                wk_sb[:, c * INNER + j * 128 : c * INNER + (j + 1) * 128],
                                ctx_sb[:, c * M : (c + 1) * M],
                                start=(c == 0),
                                stop=(c == NCHUNK_K - 1),
                            )
                        nc.scalar.copy(out=kt_sb[:, j * M : (j + 1) * M], in_=kps[:, 0:M])

                    def emit_qt_pair(nt, j):
                        qt_t.setdefault(nt, {})
                        qt_t[nt][j] = qp.tile(
                            [128, 512], F32R, tag="qt", name=f"qt{nt}_{j}"
                        )
                        qt = qt_t[nt][j]
                        xt = xt_t[nt]
                        qps = ps_a.tile([128, 512], F32, tag="psA")
                        for c in range(NCHUNK_Q):
                            nc.tensor.matmul(
                                qps[:],
                                wq_sb[:, c * INNER + j * 128 : c * INNER + (j + 1) * 128],
                                xt[:, c * 512 : (c + 1) * 512],
                                start=(c == 0),
                                stop=(c == NCHUNK_Q - 1),
                            )
                        nc.scalar.copy(out=qt[:], in_=qps[:])

                    def emit_v_group(mc, hf):
                        # V natural [m, d]: v_sb block (mc, h) cols 0..63, bf16
                        vps = ps_a.tile([128, 512], F32, tag="psA")
                        for c in range(NCHUNK_K):
                            nc.tensor.matmul(
                                vps[:],
                                ctx_sb[:, c * M + mc * 128 : c * M + (mc + 1) * 128],
                                wv_sb[:, c * INNER + hf * 512 : c * INNER + (hf + 1) * 512],
                                start=(c == 0),
                                stop=(c == NCHUNK_K - 1),
                            )
                        base = mc * H * VBLK + hf * 8 * VBLK
                        nc.vector.tensor_copy(
                            v_sb[:, base : base + 8 * VBLK].rearrange(
                                "p (h q) -> p h q", q=VBLK
                            )[:, :, 0:D],
                            vps[:].rearrange("p (h q) -> p h q", q=D),
                        )

                    # ---- phase B: software-pipelined across 512-query tiles.
                    # Scores are emitted one pair ahead (so ACT exp never
                    # paces the PE); QT-projection groups are spread by a
                    # static schedule so every pair has ~6us of PE work.
                    ets_t = {}

                    def emit_score_group(nt, j, g):
                        # g = half*2 + mcp; ets_t[(nt,j)][half][mcp] =
                        # exp(scores) of head 2j+half for m-chunks
                        # (2mcp, 2mcp+1), [128 m, 1024 n] bf16. Emitted one
                        # pair ahead so ACT exp never paces the PE.
                        half, mcp = g // 2, g % 2
                        if g == 0:
                            ets_t[(nt, j)] = [[], []]
                        qt = qt_t[nt][j]
                        p0, p1 = half * 64, half * 64 + 64
                        scps = ps_sc.tile([128, 1024], F32, tag="sc")
                        for k in range(2):
                            mc = 2 * mcp + k
                            nc.tensor.matmul(
                                scps[:, k * 512 : (k + 1) * 512],
                                kt_sb[p0:p1, j * M + mc * 128 : j * M + (mc + 1) * 128],
                                qt[p0:p1, :],
                                start=True,
                                stop=True,
                            )
                        et = ep.tile([128, 1024], BF16, tag="et")
                        nc.scalar.activation(
                            et[:], scps[:], mybir.ActivationFunctionType.Exp
                        )
                        ets_t[(nt, j)][half].append(et)

                    def emit_pv_passes(pv, ets_h, h, mcs):
                        # mc-major passes over the 4 n-chunk regions so each
                        # exp unlocks work ASAP. PSUM zero regions are the
                        # whole 2KB bank: start only on the bank's first
                        # matmul; later first-touches of pending bytes
                        # overwrite, the rest accumulate (hence
                        # skip_group_check).
                        for mc in mcs:
                            et = ets_h[mc // 2]
                            for cchunk in range(4):
                                nc.tensor.matmul(
                                    pv[:, cchunk * VBLK : (cchunk + 1) * VBLK],
                                    et[:, (mc % 2) * 512 + cchunk * 128 : (mc % 2) * 512 + (cchunk + 1) * 128],
                                    v_sb[:, mc * H * VBLK + h * VBLK : mc * H * VBLK + (h + 1) * VBLK],
                                    start=(mc == 0 and cchunk == 0),
                                    stop=(mc == NMC - 1 and cchunk == 3),
                                    skip_group_check=True,
                                )

                    def emit_norm_half(pv, half, ot_nd):
                        # recip of the denominator column (64::65), then
                        # normalize during the psum->sbuf copy (per-partition
                        # scalar = per-query), bf16 out
                        rrec = rp.tile([128, 4], F32, tag="rr")
                        nc.vector.reciprocal(
                            rrec[:].rearrange("p (c q) -> p c q", q=1),
                            pv[:, 0 : 4 * VBLK].rearrange("p (c q) -> p c q", q=VBLK)[
                                :, :, D : D + 1
                            ],
                        )
                        for cchunk in range(4):
                            nc.vector.tensor_scalar_mul(
                                ot_nd[:, cchunk * 128 + half * 64 : cchunk * 128 + half * 64 + 64],
                                pv[:, cchunk * VBLK : cchunk * VBLK + D],
                                rrec[:, cchunk : cchunk + 1],
                            )

                    def emit_transpose(nt, j, ot_nd):
                        # ot_nd [128 n, 128 dd] chunks -> otT [128 dd, 512 n]
                        tps = ps_a.tile([128, 512], BF16, tag="psA")
                        for cchunk in range(4):
                            nc.tensor.matmul(
                                tps[:, cchunk * 128 : (cchunk + 1) * 128],
                                ot_nd[:, cchunk * 128 : (cchunk + 1) * 128],
                                ident[:],
                                is_transpose=True,
                                start=(cchunk == 0),
                                stop=(cchunk == 3),
                                skip_group_check=True,
                            )
                        if j == 0:
                            otT_t[nt] = {}
                        otT_t[nt][j] = otp.tile(
                            [128, 512], BF16, tag="otT", name=f"otT{nt}_{j}"
                        )
                        nc.vector.tensor_copy(otT_t[nt][j][:], tps[:])

                    def emit_wo_group(nt, g, pool=None, psum_bias=False, split=False):
                        # psum_bias=True: seed yps with the bias via a PE
                        # outer-product matmul and copy via ACT (idle at the
                        # tail), skipping the DVE add. split=True additionally
                        # runs two column-half chains so the first half's
                        # copy+DMA overlap the second half's matmuls.
                        ns, cg = g // 2, g % 2
                        yps = (pool or ps_a).tile(
                            [128, 512], F32, tag="psA" if pool is None else "pv"
                        )
                        if psum_bias:
                            nc.tensor.matmul(
                                yps[:],
                                ones_bf[0:1, 0:128],
                                bo_bf[0:1, cg * 512 : (cg + 1) * 512],
                                start=True,
                                stop=False,
                            )
                        for j in range(NPAIR):
                            nc.tensor.matmul(
                                yps[:],
                                otT_t[nt][j][:, ns * 128 : (ns + 1) * 128],
                                wo_sb[:, j * C + cg * 512 : j * C + (cg + 1) * 512],
                                start=(j == 0 and not psum_bias),
                                stop=(j == NPAIR - 1),
                            )
                        ysb = yp.tile([128, 512], BF16, tag="ysb")
                        if psum_bias:
                            # bias already seeded in psum; ACT is idle at the
                            # tail while DVE drains copybacks — use it
                            nc.scalar.copy(out=ysb[:], in_=yps[:])
                        else:
                            nc.vector.tensor_add(
                                ysb[:], yps[:], bo_sb[:, cg * 512 : (cg + 1) * 512]
                            )
                        nc.sync.dma_start(
                            out=y[
                                nt * 512 + ns * 128 : nt * 512 + (ns + 1) * 128,
                                cg * 512 : (cg + 1) * 512,
                            ],
                            in_=ysb[:],
                        )

                    # Static QT-projection schedule: pair (nt, j) -> list of
                    # (target_nt, target_j) groups to emit there. qt(t, j)
                    # must land >= 1 pair before its sc lookahead at
                    # (t, j-1). nt3 carries none (its ACT is exp-only).
                    qt_sched = {}
                    for j in range(NPAIR):
                        qt_sched[(0, j)] = []
                        if j < 6:
                            qt_sched[(0, j)].append((0, j + 2))
                        qt_sched[(0, j)].append((1, j))
                        if j >= 6:
                            qt_sched[(0, j)].append((2, j - 6))
                        qt_sched[(1, j)] = [(2, j + 2)] if j < 6 else [(3, j - 6)]
                        qt_sched[(2, j)] = [(3, j + 2)] if j < 6 else []
                        qt_sched[(3, j)] = []

                    # bootstrap: K/V projections while weights stream in,
                    # then the first two QT pairs and pair (0,0)'s scores
                    for mc in range(NMC):
                        emit_v_group(mc, 0)
                    emit_qt_pair(0, 0)
                    emit_qt_pair(0, 1)
                    for g in range(4):
                        emit_score_group(0, 0, g)

                    prev = None  # (nt, j, ot_nd) awaiting transpose

                    for nt in range(NT):
                        for j in range(NPAIR):
                            nxt = (
                                (nt, j + 1)
                                if j + 1 < NPAIR
                                else ((nt + 1, 0) if nt + 1 < NT else None)
                            )
                            if j == 0 and nt + 2 < NT:
                                emit_x_dma(nt + 2)
                            # short pairs (no qt fill): T after Wo, Wo after
                            # norm (but T(nt-1, 7) must precede the first
                            # wo(nt-1, *) group)
                            late_t = (nt == NT - 1 and j > 0) or (
                                nt == NT - 2 and j >= 6
                            )
                            late_wo = late_t or (nt == NT - 1 and j == 0)
                            ets = ets_t.pop((nt, j))
                            pv0 = ps_pv.tile([128, 512], F32, tag="pv")
                            pv1 = ps_pv.tile([128, 512], F32, tag="pv")
                            h0, h1 = 2 * j, 2 * j + 1
                            ot_nd = onp.tile([128, 512], BF16, tag="ond")
                            pv0_first = nt == NT - 1
                            if pv0_first:
                                # short pairs: pv0 first (its exps finished a
                                # pair ago), delaying the sc allocations so
                                # they clear the predecessor's exp WARs
                                emit_pv_passes(pv0, ets[0], h0, [0, 1, 2, 3])
                                emit_norm_half(pv0, 0, ot_nd)
                            if nxt:
                                emit_score_group(*nxt, 0)
                                emit_score_group(*nxt, 1)
                            if prev is not None and not late_t:
                                emit_transpose(*prev)
                            if nxt:
                                emit_score_group(*nxt, 2)
                            if nt > 0 and not late_wo:
                                emit_wo_group(nt - 1, j)
                            if prev is not None and late_t:
                                emit_transpose(*prev)
                            if nxt:
                                emit_score_group(*nxt, 3)
                            if nt == 0 and j < NMC:
                                emit_v_group(j, 1)
                            if not pv0_first:
                                emit_pv_passes(pv0, ets[0], h0, [0, 1, 2, 3])
                                emit_norm_half(pv0, 0, ot_nd)
                            emit_pv_passes(pv1, ets[1], h1, [0, 1, 2, 3])
                            emit_norm_half(pv1, 1, ot_nd)
                            if late_wo:
                                # wo after norm: the DVE runs the scale
                                # copies (which gate the next transpose)
                                # before the ysb add
                                emit_wo_group(nt - 1, j)
                            qts = qt_sched[(nt, j)]
                            for tgt in qts:
                                emit_qt_pair(*tgt)
                            prev = (nt, j, ot_nd)
                    emit_transpose(*prev)
                    # tail: pv banks are free; alternate them in as extra
                    # yps slots so the ysb-add chain never blocks the PE
                    for g in range(NPAIR):
                        emit_wo_group(
                            NT - 1,
                            g,
                            pool=ps_pv if g % 2 else None,
                            psum_bias=(g >= NPAIR - 2),
                        )

    nc.compile()
    return nc


_NC_CACHE = None


def kernel(x, context, Wq, Wk, Wv, Wo, bo, _trace=False, _trace_kwargs=None):
    global _NC_CACHE
    if _NC_CACHE is None:
        _NC_CACHE = build_nc()
    nc = _NC_CACHE

    bf16 = ml_dtypes.bfloat16
    x = np.asarray(x, np.float32)
    context = np.asarray(context, np.float32)
    wq_s = (np.asarray(Wq, np.float32) * np.float32(D**-0.5)).astype(bf16)
    wk = np.asarray(Wk, np.float32).astype(bf16)
    wv = np.asarray(Wv, np.float32).astype(bf16)
    wo = np.asarray(Wo, np.float32).astype(bf16)
    bo2 = np.asarray(bo, np.float32).reshape(1, C)

    in_maps = []
    for i in range(8):
        b, hf = i // 2, i % 2
        in_maps.append(
            {
                "xT": np.ascontiguousarray(
                    x[b, hf * NPC : (hf + 1) * NPC, :].T
                ).astype(bf16),
                "ctxT": np.ascontiguousarray(context[b].T).astype(bf16),
                "wq": wq_s,
                "wk": wk,
                "wv": wv,
                "wo": wo,
                "bo": bo2,
            }
        )

    kw = {}
    if _trace:
        kw = dict(trace=True, trace_kwargs=_trace_kwargs or {})
    res = run_bass_kernel_spmd(nc, in_maps, list(range(8)), **kw)

    out = np.empty((B, N, C), np.float32)
    for i in range(8):
        b, hf = i // 2, i % 2
        out[b, hf * NPC : (hf + 1) * NPC, :] = np.asarray(
            res.results[i]["y"], np.float32
        )
    if _trace:
        return out, res
    return out
